# revision 2
# baseline (speedup 1.0000x reference)
"""MoE (2-expert SwiGLU MLP with token routing) on 8 Trainium2 NeuronCores.

Strategy: host-side routing + data-parallel dense MLP.
  - Sort tokens by expert (stable argsort of routing_mask).
  - Pack tokens into fixed "slots": 32 slots of 512 tokens + 8 slots of 128
    tokens (4x512 + 1x128 per core). Each slot is single-expert; the slot's
    expert weights are passed as that pass's runtime input tensors, so one
    SPMD program handles any expert split. Total capacity 17408 token-slots
    covers the worst-case padding of 16384 tokens (proof in _plan_slots).
  - Per pass the kernel computes hT = silu(wg^T x) * (wu^T x) in transposed
    layout [FF, T] so the down projection contracts FF entirely inside PSUM
    (out[dT] accumulated over 44 ff chunks), with bf16 matmuls and fp32
    accumulation.
"""

import math

import numpy as np
import ml_dtypes

import concourse.mybir as mybir
from concourse import bacc
from concourse.tile import TileContext
from concourse.bass_utils import run_bass_kernel_spmd

B, S, D, FF = 4, 4096, 2048, 5632
N = B * S
NCORES = 8
P = 128
KC = D // P    # 16 contraction chunks for gate/up
FC = FF // P   # 44 ff chunks
DC = D // P    # 16 output d chunks
T512 = 512
T128 = 128
NP512 = 4      # 512-token passes per core
NPASS = NP512 + 1
BF16 = ml_dtypes.bfloat16

_CACHE: dict = {}


def _build_nc():
    nc = bacc.Bacc("TRN2", target_bir_lowering=False, debug=False, num_devices=NCORES)
    bf = mybir.dt.bfloat16
    f32 = mybir.dt.float32

    x512 = nc.declare_dram_parameter("x512", [NP512, P, KC, T512], bf, isOutput=False)
    x128 = nc.declare_dram_parameter("x128", [P, KC, T128], bf, isOutput=False)
    wgs = [
        nc.declare_dram_parameter(f"wg{p}", [FC, P, KC, P], bf, isOutput=False)
        for p in range(NPASS)
    ]
    wus = [
        nc.declare_dram_parameter(f"wu{p}", [FC, P, KC, P], bf, isOutput=False)
        for p in range(NPASS)
    ]
    wds = [
        nc.declare_dram_parameter(f"wd{p}", [DC, P, FC, P], bf, isOutput=False)
        for p in range(NPASS)
    ]
    o512 = nc.declare_dram_parameter("o512", [NP512, DC, P, T512], f32, isOutput=True)
    o128 = nc.declare_dram_parameter("o128", [DC, P, T128], f32, isOutput=True)

    silu = mybir.ActivationFunctionType.Silu

    with TileContext(nc) as tc:
        with (
            tc.tile_pool(name="xp", bufs=2) as xp,
            tc.tile_pool(name="wp", bufs=3) as wp,
            tc.tile_pool(name="wdp", bufs=2) as wdp,
            tc.tile_pool(name="hp", bufs=2) as hp,
            tc.tile_pool(name="sgp", bufs=3) as sgp,
            tc.tile_pool(name="op", bufs=3) as op,
            tc.tile_pool(name="pgu", bufs=2, space="PSUM") as pgu,
            tc.tile_pool(name="po", bufs=2, space="PSUM") as po,
        ):
            for p in range(NPASS):
                Tp = T512 if p < NP512 else T128
                x_ap = x512[p] if p < NP512 else x128.ap()
                wg_ap, wu_ap, wd_ap = wgs[p], wus[p], wds[p]

                x_t = xp.tile([P, KC, Tp], bf, name=f"x_{p}", tag="x")
                nc.sync.dma_start(out=x_t[:], in_=x_ap)

                h_tiles = []
                for fc in range(FC):
                    wg_t = wp.tile([P, KC, P], bf, name=f"wg_{p}_{fc}", tag="wg")
                    nc.sync.dma_start(out=wg_t[:], in_=wg_ap[fc])
                    wu_t = wp.tile([P, KC, P], bf, name=f"wu_{p}_{fc}", tag="wu")
                    nc.sync.dma_start(out=wu_t[:], in_=wu_ap[fc])

                    g_ps = pgu.tile([P, Tp], f32, name=f"g_{p}_{fc}", tag="g")
                    u_ps = pgu.tile([P, Tp], f32, name=f"u_{p}_{fc}", tag="u")
                    for k in range(KC):
                        nc.tensor.matmul(
                            g_ps[:], wg_t[:, k, :], x_t[:, k, :],
                            start=(k == 0), stop=(k == KC - 1),
                        )
                    for k in range(KC):
                        nc.tensor.matmul(
                            u_ps[:], wu_t[:, k, :], x_t[:, k, :],
                            start=(k == 0), stop=(k == KC - 1),
                        )
                    sg = sgp.tile([P, Tp], f32, name=f"sg_{p}_{fc}", tag="sg")
                    nc.scalar.activation(sg[:], g_ps[:], silu)
                    h_t = hp.tile([P, Tp], bf, name=f"h_{p}_{fc}", tag=f"h{fc}")
                    nc.vector.tensor_mul(h_t[:], sg[:], u_ps[:])
                    h_tiles.append(h_t)

                for dc in range(DC):
                    wd_t = wdp.tile([P, FC, P], bf, name=f"wd_{p}_{dc}", tag="wd")
                    nc.sync.dma_start(out=wd_t[:], in_=wd_ap[dc])
                    o_ps = po.tile([P, Tp], f32, name=f"o_{p}_{dc}", tag="o")
                    for fc in range(FC):
                        nc.tensor.matmul(
                            o_ps[:], wd_t[:, fc, :], h_tiles[fc][:],
                            start=(fc == 0), stop=(fc == FC - 1),
                        )
                    o_sb = op.tile([P, Tp], f32, name=f"os_{p}_{dc}", tag="os")
                    nc.vector.tensor_copy(o_sb[:], o_ps[:])
                    o_ap = o512[p, dc] if p < NP512 else o128[dc]
                    nc.sync.dma_start(out=o_ap, in_=o_sb[:])

    nc.compile()
    return nc


def _get_nc():
    if "nc" not in _CACHE:
        _CACHE["nc"] = _build_nc()
    return _CACHE["nc"]


def _plan_slots(n0: int, n1: int):
    """Assign sorted tokens to 32x512 + 8x128 single-expert slots.

    u_e = ceil(n_e/128) 128-token units; expert e uses a_e = u_e//4 full
    512-slots then b_e = u_e%4 128-slots. Feasibility for any split of
    N=16384: u0+u1 <= N/128 + 2 = 130, so a0+a1 <= 32 and b0+b1 <= 6.
    Returns (slots512, slots128): lists of (expert, tok_start, ntok).
    """
    slots512 = []
    slots128 = []
    tok = 0
    for e, ne in ((0, n0), (1, n1)):
        u = -(-ne // P)
        a, b = u // 4, u % 4
        rem = ne
        for _ in range(a):
            t = min(T512, rem)
            slots512.append((e, tok, t))
            tok += t
            rem -= t
        for _ in range(b):
            t = min(T128, rem)
            slots128.append((e, tok, t))
            tok += t
            rem -= t
        assert rem == 0
    assert len(slots512) <= 32 and len(slots128) <= 8, (n0, n1)
    while len(slots512) < 32:
        slots512.append((0, 0, 0))
    while len(slots128) < 8:
        slots128.append((0, 0, 0))
    return slots512, slots128


def _block_weights(wg, wu, wd):
    """Blocked bf16 layouts for one expert (contiguous per DMA tile)."""
    wgR = np.ascontiguousarray(
        wg.reshape(KC, P, FC, P).transpose(2, 1, 0, 3).astype(BF16)
    )
    wuR = np.ascontiguousarray(
        wu.reshape(KC, P, FC, P).transpose(2, 1, 0, 3).astype(BF16)
    )
    wdR = np.ascontiguousarray(
        wd.reshape(FC, P, DC, P).transpose(2, 1, 0, 3).astype(BF16)
    )
    return wgR, wuR, wdR


def kernel(hidden_states, routing_mask, w_gate, w_up, w_down):
    x = np.asarray(hidden_states, dtype=np.float32).reshape(N, D)
    mask = np.asarray(routing_mask).reshape(N)
    w_gate = np.asarray(w_gate, dtype=np.float32)
    w_up = np.asarray(w_up, dtype=np.float32)
    w_down = np.asarray(w_down, dtype=np.float32)

    is_e1 = (mask != 0).astype(np.int32)
    perm = np.argsort(is_e1, kind="stable")
    n1 = int(is_e1.sum())
    n0 = N - n1

    slots512, slots128 = _plan_slots(n0, n1)

    wR = [_block_weights(w_gate[e], w_up[e], w_down[e]) for e in range(2)]
    x_sorted = x[perm].astype(BF16)

    in_maps = []
    for c in range(NCORES):
        m = {}
        x5 = np.zeros((NP512, P, KC, T512), dtype=BF16)
        for p in range(NP512):
            e, t0, nt = slots512[c * NP512 + p]
            if nt > 0:
                blk = np.zeros((T512, D), dtype=BF16)
                blk[:nt] = x_sorted[t0:t0 + nt]
                x5[p] = blk.reshape(T512, KC, P).transpose(2, 1, 0)
            m[f"wg{p}"], m[f"wu{p}"], m[f"wd{p}"] = wR[e]
        e, t0, nt = slots128[c]
        x1 = np.zeros((P, KC, T128), dtype=BF16)
        if nt > 0:
            blk = np.zeros((T128, D), dtype=BF16)
            blk[:nt] = x_sorted[t0:t0 + nt]
            x1 = np.ascontiguousarray(blk.reshape(T128, KC, P).transpose(2, 1, 0))
        m["wg4"], m["wu4"], m["wd4"] = wR[e]
        m["x512"] = x5
        m["x128"] = x1
        in_maps.append(m)

    nc = _get_nc()
    res = run_bass_kernel_spmd(nc, in_maps, core_ids=list(range(NCORES)))

    out_sorted = np.zeros((N, D), dtype=np.float32)
    for s, (e, t0, nt) in enumerate(slots512):
        if nt == 0:
            continue
        c, p = divmod(s, NP512)
        blk = res.results[c]["o512"][p]  # [DC, P, T512]
        out_sorted[t0:t0 + nt] = blk.transpose(2, 0, 1).reshape(T512, D)[:nt]
    for c, (e, t0, nt) in enumerate(slots128):
        if nt == 0:
            continue
        blk = res.results[c]["o128"]  # [DC, P, T128]
        out_sorted[t0:t0 + nt] = blk.transpose(2, 0, 1).reshape(T128, D)[:nt]

    out = np.zeros((N, D), dtype=np.float32)
    out[perm] = out_sorted
    return out.reshape(B, S, D)


# revision 3
# speedup vs baseline: 1.0439x; 1.0439x over previous
"""MoE (2-expert SwiGLU MLP with token routing) on 8 Trainium2 NeuronCores.

Strategy: host-side routing + data-parallel dense MLP.
  - Sort tokens by expert (stable argsort of routing_mask).
  - Main work: 32 slots of 512 tokens (4 passes per core), each slot
    single-expert; the slot's expert weights are that pass's runtime input
    tensors, so one SPMD program handles any expert split.
  - Tail (up to 384 leftover tokens per expert after 512-slot packing):
    processed FF-sharded — every core computes a 768-wide FF slice of BOTH
    experts for all tail tokens; the host sums the 8 partial outputs.
    This keeps the tail's weight traffic per core small enough to hide
    under compute (a full-FF tail pass is weight-DMA-bound).
  - Per pass the kernel computes hT = silu(wg^T x) * (wu^T x) in transposed
    layout [FF, T] so the down projection contracts FF entirely inside PSUM,
    with fp16 matmuls and fp32 accumulation.

Feasibility for any mask split (N = 16384 tokens): with u_e = ceil(N_e/128)
128-token units, expert e uses a_e = u_e//4 512-slots and b_e = u_e%4 tail
units. a_0+a_1 <= 32 and b_e <= 3 (tail <= 384 tokens/expert) always hold.
"""

import numpy as np

import concourse.mybir as mybir
from concourse import bacc
from concourse.tile import TileContext
from concourse.bass_utils import run_bass_kernel_spmd

B, S, D, FF = 4, 4096, 2048, 5632
N = B * S
NCORES = 8
P = 128
KC = D // P     # 16 contraction chunks for gate/up
FC = FF // P    # 44 ff chunks
DC = D // P     # 16 output d chunks
T512 = 512
NP512 = 4       # 512-token passes per core
TTAIL = 384     # tail capacity per expert
FFS = 768       # per-core ff slice in the tail pass (8*768 >= FF, zero-padded)
FCS = FFS // P  # 6
F16 = np.float16

_CACHE: dict = {}


def _build_nc():
    nc = bacc.Bacc("TRN2", target_bir_lowering=False, debug=False, num_devices=NCORES)
    f16 = mybir.dt.float16
    f32 = mybir.dt.float32

    x512 = nc.declare_dram_parameter("x512", [NP512, P, KC, T512], f16, isOutput=False)
    wgs = [
        nc.declare_dram_parameter(f"wg{p}", [FC, P, KC, P], f16, isOutput=False)
        for p in range(NP512)
    ]
    wus = [
        nc.declare_dram_parameter(f"wu{p}", [FC, P, KC, P], f16, isOutput=False)
        for p in range(NP512)
    ]
    wds = [
        nc.declare_dram_parameter(f"wd{p}", [DC, P, FC, P], f16, isOutput=False)
        for p in range(NP512)
    ]
    xt = nc.declare_dram_parameter("xt", [2, P, KC, TTAIL], f16, isOutput=False)
    wgt = nc.declare_dram_parameter("wgt", [2, FCS, P, KC, P], f16, isOutput=False)
    wut = nc.declare_dram_parameter("wut", [2, FCS, P, KC, P], f16, isOutput=False)
    wdt = nc.declare_dram_parameter("wdt", [2, DC, P, FCS, P], f16, isOutput=False)
    o512 = nc.declare_dram_parameter("o512", [NP512, DC, P, T512], f32, isOutput=True)
    ot = nc.declare_dram_parameter("ot", [2, DC, P, TTAIL], f32, isOutput=True)

    silu = mybir.ActivationFunctionType.Silu

    with TileContext(nc) as tc:
        with (
            tc.tile_pool(name="xp", bufs=2) as xp,
            tc.tile_pool(name="wp", bufs=3) as wp,
            tc.tile_pool(name="wdp", bufs=2) as wdp,
            tc.tile_pool(name="hp", bufs=2) as hp,
            tc.tile_pool(name="sgp", bufs=3) as sgp,
            tc.tile_pool(name="op", bufs=3) as op,
            tc.tile_pool(name="pgu", bufs=2, space="PSUM") as pgu,
            tc.tile_pool(name="po", bufs=2, space="PSUM") as po,
        ):
            def mlp_pass(tag, Tp, nfc, x_t, wg_ap, wu_ap, wd_ap, o_ap):
                """One dense SwiGLU pass: x_t [P, KC, Tp] SBUF tile;
                wg/wu_ap[fc] -> [P, KC, P]; wd_ap[dc] -> [P, nfc, P];
                o_ap(dc) -> [P, Tp] DRAM."""
                h_tiles = []
                for fc in range(nfc):
                    wg_t = wp.tile([P, KC, P], f16, name=f"wg_{tag}_{fc}", tag="wg")
                    nc.sync.dma_start(out=wg_t[:], in_=wg_ap[fc])
                    wu_t = wp.tile([P, KC, P], f16, name=f"wu_{tag}_{fc}", tag="wu")
                    nc.sync.dma_start(out=wu_t[:], in_=wu_ap[fc])

                    g_ps = pgu.tile([P, Tp], f32, name=f"g_{tag}_{fc}", tag="g")
                    u_ps = pgu.tile([P, Tp], f32, name=f"u_{tag}_{fc}", tag="u")
                    for k in range(KC):
                        nc.tensor.matmul(
                            g_ps[:], wg_t[:, k, :], x_t[:, k, :],
                            start=(k == 0), stop=(k == KC - 1),
                        )
                    for k in range(KC):
                        nc.tensor.matmul(
                            u_ps[:], wu_t[:, k, :], x_t[:, k, :],
                            start=(k == 0), stop=(k == KC - 1),
                        )
                    sg = sgp.tile([P, Tp], f32, name=f"sg_{tag}_{fc}", tag="sg")
                    nc.scalar.activation(sg[:], g_ps[:], silu)
                    h_t = hp.tile([P, Tp], f16, name=f"h_{tag}_{fc}", tag=f"h{fc}")
                    nc.vector.tensor_mul(h_t[:], sg[:], u_ps[:])
                    h_tiles.append(h_t)

                for dc in range(DC):
                    wd_t = wdp.tile([P, nfc, P], f16, name=f"wd_{tag}_{dc}", tag="wd")
                    nc.sync.dma_start(out=wd_t[:], in_=wd_ap[dc])
                    o_ps = po.tile([P, Tp], f32, name=f"o_{tag}_{dc}", tag="o")
                    for fc in range(nfc):
                        nc.tensor.matmul(
                            o_ps[:], wd_t[:, fc, :], h_tiles[fc][:],
                            start=(fc == 0), stop=(fc == nfc - 1),
                        )
                    o_sb = op.tile([P, Tp], f32, name=f"os_{tag}_{dc}", tag="os")
                    nc.vector.tensor_copy(o_sb[:], o_ps[:])
                    nc.sync.dma_start(out=o_ap(dc), in_=o_sb[:])

            for p in range(NP512):
                x_t = xp.tile([P, KC, T512], f16, name=f"x_{p}", tag="x")
                nc.sync.dma_start(out=x_t[:], in_=x512[p])
                mlp_pass(
                    f"m{p}", T512, FC, x_t,
                    wgs[p], wus[p], wds[p],
                    lambda dc, p=p: o512[p, dc],
                )

            for e in range(2):
                xt_t = xp.tile([P, KC, TTAIL], f16, name=f"xt_{e}", tag="x")
                nc.sync.dma_start(out=xt_t[:], in_=xt[e])
                mlp_pass(
                    f"t{e}", TTAIL, FCS, xt_t,
                    wgt[e], wut[e], wdt[e],
                    lambda dc, e=e: ot[e, dc],
                )

    nc.compile()
    return nc


def _get_nc():
    if "nc" not in _CACHE:
        _CACHE["nc"] = _build_nc()
    return _CACHE["nc"]


def _plan_slots(n0: int, n1: int):
    """Pack sorted tokens into 32 x 512-token single-expert slots plus a
    per-expert tail of <= 384 tokens. Returns (slots512, tails) where
    slots512 is 32 x (expert, tok_start, ntok) and tails is
    [(tok_start, ntok)] * 2 indexed by expert."""
    slots512 = []
    tails = []
    tok = 0
    for e, ne in ((0, n0), (1, n1)):
        u = -(-ne // P)
        a, b = u // 4, u % 4
        rem = ne
        for _ in range(a):
            t = min(T512, rem)
            slots512.append((e, tok, t))
            tok += t
            rem -= t
        assert rem <= b * P <= TTAIL
        tails.append((tok, rem))
        tok += rem
    assert len(slots512) <= 32, (n0, n1)
    while len(slots512) < 32:
        slots512.append((0, 0, 0))
    return slots512, tails


def _block_weights(wg, wu, wd):
    """Blocked fp16 layouts for one expert's main passes."""
    wgR = np.ascontiguousarray(
        wg.reshape(KC, P, FC, P).transpose(2, 1, 0, 3).astype(F16)
    )
    wuR = np.ascontiguousarray(
        wu.reshape(KC, P, FC, P).transpose(2, 1, 0, 3).astype(F16)
    )
    wdR = np.ascontiguousarray(
        wd.reshape(FC, P, DC, P).transpose(2, 1, 0, 3).astype(F16)
    )
    return wgR, wuR, wdR


def _block_x(tokens, Tp):
    """[ntok<=Tp, D] fp16 -> [P, KC, Tp] blocked (zero-padded)."""
    blk = np.zeros((Tp, D), dtype=F16)
    blk[: tokens.shape[0]] = tokens
    return blk.reshape(Tp, KC, P).transpose(2, 1, 0)


def kernel(hidden_states, routing_mask, w_gate, w_up, w_down):
    x = np.asarray(hidden_states, dtype=np.float32).reshape(N, D)
    mask = np.asarray(routing_mask).reshape(N)
    w_gate = np.asarray(w_gate, dtype=np.float32)
    w_up = np.asarray(w_up, dtype=np.float32)
    w_down = np.asarray(w_down, dtype=np.float32)

    is_e1 = (mask != 0).astype(np.int32)
    perm = np.argsort(is_e1, kind="stable")
    n1 = int(is_e1.sum())
    n0 = N - n1

    slots512, tails = _plan_slots(n0, n1)

    wR = [_block_weights(w_gate[e], w_up[e], w_down[e]) for e in range(2)]
    x_sorted = x[perm].astype(F16)

    # tail inputs: shared across cores (x), per-core ff slice (weights)
    xt_arr = np.zeros((2, P, KC, TTAIL), dtype=F16)
    for e, (t0, nt) in enumerate(tails):
        if nt > 0:
            xt_arr[e] = _block_x(x_sorted[t0:t0 + nt], TTAIL)
    wgP = np.zeros((2, D, NCORES * FFS), dtype=np.float32)
    wuP = np.zeros((2, D, NCORES * FFS), dtype=np.float32)
    wdP = np.zeros((2, NCORES * FFS, D), dtype=np.float32)
    wgP[:, :, :FF] = w_gate
    wuP[:, :, :FF] = w_up
    wdP[:, :FF, :] = w_down

    in_maps = []
    for c in range(NCORES):
        m = {}
        x5 = np.zeros((NP512, P, KC, T512), dtype=F16)
        for p in range(NP512):
            e, t0, nt = slots512[c * NP512 + p]
            if nt > 0:
                x5[p] = _block_x(x_sorted[t0:t0 + nt], T512)
            m[f"wg{p}"], m[f"wu{p}"], m[f"wd{p}"] = wR[e]
        m["x512"] = x5
        m["xt"] = xt_arr
        sl = slice(c * FFS, (c + 1) * FFS)
        m["wgt"] = np.ascontiguousarray(
            wgP[:, :, sl].reshape(2, KC, P, FCS, P).transpose(0, 3, 2, 1, 4).astype(F16)
        )
        m["wut"] = np.ascontiguousarray(
            wuP[:, :, sl].reshape(2, KC, P, FCS, P).transpose(0, 3, 2, 1, 4).astype(F16)
        )
        m["wdt"] = np.ascontiguousarray(
            wdP[:, sl, :].reshape(2, FCS, P, DC, P).transpose(0, 3, 2, 1, 4).astype(F16)
        )
        in_maps.append(m)

    nc = _get_nc()
    res = run_bass_kernel_spmd(nc, in_maps, core_ids=list(range(NCORES)))

    out_sorted = np.zeros((N, D), dtype=np.float32)
    for s, (e, t0, nt) in enumerate(slots512):
        if nt == 0:
            continue
        c, p = divmod(s, NP512)
        blk = res.results[c]["o512"][p]  # [DC, P, T512]
        out_sorted[t0:t0 + nt] = blk.transpose(2, 0, 1).reshape(T512, D)[:nt]
    ot_sum = np.zeros((2, DC, P, TTAIL), dtype=np.float32)
    for c in range(NCORES):
        ot_sum += res.results[c]["ot"]
    for e, (t0, nt) in enumerate(tails):
        if nt > 0:
            out_sorted[t0:t0 + nt] = (
                ot_sum[e].transpose(2, 0, 1).reshape(TTAIL, D)[:nt]
            )

    out = np.zeros((N, D), dtype=np.float32)
    out[perm] = out_sorted
    return out.reshape(B, S, D)
